# revision 1
# baseline (speedup 1.0000x reference)
"""Trainium2 Bass kernel for nn_DistanceLoss (patch neighbor-distance loss).

Reference semantics (k=16, H=W=2048, LOSS_WEIGHT=1):
  split each image into non-overlapping 16x16 patches; for interior pixels
  (local i,j in 1..14) and the 8-neighbor offset list [E,NW,NE,N,E,SW,SE,S]
  (E twice, W missing), accumulate || |sr_c-sr_n| - |hr_c-hr_n| || and take
  the global mean over L*14*14*8 terms.

Identity: for u = sr_c-sr_n, v = hr_c-hr_n,
    ||u|-|v|| = min(|u+v|, |u-v|) = min(|S_c-S_n|, |D_c-D_n|)
with S = sr+hr, D = sr-hr. Opposite offsets +o/-o share one difference
array t: sum_I t(f,-o) = sum_{I-o} t(g,+o), so the pairs {N,S}, {NW,SE},
{NE,SW} cost one elementwise pass each; E (listed twice) has weight 2.

Sharding: 256 image columns per core (16 patch-cols x 128 patch-rows).
Host reshapes each slab to [128, 4096] (partition = patch-row, free =
i*256+c) making every neighbor offset the constant free shift di*256+dj
and the DMA fully contiguous.

Engines: DVE computes p|q = SD - SD_shift (stacked S|D tile, one 2x TT),
|x| via int16 sign-bit clear (4x TS; one pair on DVE, three as ACT Abs),
and t = min(|p|,|q|). The interior-window sums run on the otherwise-idle
PE as ones/twos-weighted [128,1]^T @ t-row matmuls accumulating into a
single PSUM [1,224] region - the per-row weights {1,2,...,2,1} encode
both shifted reduction windows of each offset pair, edge strips get
weight-1 matmuls, and E bakes its x2. One tiny reduce drains PSUM to a
scalar. Shifted copies SDo = SD[:,1:] ride an idle SBUF->SBUF DMA; input
loads are HWDGE fp32 in 4 chunks overlapped with the S/D TTs.
"""

import numpy as np

H = W = 2048
K = 16
NCORES = 8
WC = W // NCORES          # 256 columns per core
FREE = K * WC             # 4096 free elements per partition
WIN = 15 * WC             # 3840: compute window covers i = 0..14
PADW = 3904               # t tile width (views may overrun WIN slightly)
PQW = 2 * PADW            # stacked p|q tile width
NCHUNK = 4                # input-DMA chunks for load/compute overlap
N_TERMS = (H // K) * (W // K) * (K - 2) * (K - 2) * 8


def _split_multiwaits(nc):
    """The walrus build here accepts at most one sync wait (and one update)
    per instruction: hoist extra waits onto same-engine NoOps inserted
    before the instruction, and extra updates onto NoOps after it."""
    from concourse import mybir

    k = 0
    for f in nc.m.functions:
        for bb in f.blocks:
            out, changed = [], False
            for i in bb.instructions:
                si = i.sync_info
                waits = list(si.on_wait) if si else []
                ups = list(si.on_update) if si else []
                trimmed = False
                if len(waits) > 1:
                    for w in waits[:-1]:
                        n = mybir.InstNoOp(name=f"{i.name}-sw{k}", ins=[],
                                           outs=[])
                        k += 1
                        n.engine = i.engine
                        n.sync_info = mybir.SyncInfo(on_wait=[w], on_update=[])
                        out.append(n)
                    waits, changed, trimmed = waits[-1:], True, True
                out.append(i)
                if len(ups) > 1:
                    i.sync_info = mybir.SyncInfo(on_wait=waits,
                                                 on_update=ups[:1])
                    for u in ups[1:]:
                        n = mybir.InstNoOp(name=f"{i.name}-su{k}", ins=[],
                                           outs=[])
                        k += 1
                        n.engine = i.engine
                        n.sync_info = mybir.SyncInfo(on_wait=[], on_update=[u])
                        out.append(n)
                    changed = True
                elif trimmed:
                    i.sync_info = mybir.SyncInfo(on_wait=waits, on_update=ups)
            if changed:
                bb.instructions = out
    return k


def _build_bass():
    from concourse import bass, mybir, tile

    nc = bass.Bass()
    x_sr = nc.declare_dram_parameter("x_sr", [128, FREE], mybir.dt.float16,
                                     isOutput=False)
    x_hr = nc.declare_dram_parameter("x_hr", [128, FREE], mybir.dt.float16,
                                     isOutput=False)
    out_sum = nc.declare_dram_parameter("out_sum", [1, 8],
                                        mybir.dt.float32, isOutput=True)

    fp16 = mybir.dt.float16
    f32 = mybir.dt.float32
    Alu = mybir.AluOpType
    Act = mybir.ActivationFunctionType

    with tile.TileContext(nc) as tc:
        with tc.tile_pool(name="io", bufs=1) as io_pool, \
             tc.tile_pool(name="sd", bufs=1) as sd_pool, \
             tc.tile_pool(name="pq", bufs=3) as pq_pool, \
             tc.tile_pool(name="tpool", bufs=4) as t_pool, \
             tc.tile_pool(name="psum", bufs=1, space="PSUM") as psum_pool:
            sr_t = io_pool.tile([128, FREE], fp16, tag="sr")
            hr_t = io_pool.tile([128, FREE], fp16, tag="hr")
            SD = sd_pool.tile([128, 2 * FREE], fp16, tag="SD")
            SDo = sd_pool.tile([128, 2 * FREE], fp16, tag="SDo")
            w1 = sd_pool.tile([128, 1], fp16, tag="w1")
            w2 = sd_pool.tile([128, 1], fp16, tag="w2")
            acc = psum_pool.tile([1, 256], f32, tag="acc")
            colsb = sd_pool.tile([1, 8], f32, tag="colsb")

            nc.vector.memset(w1[:, :], 1.0)
            nc.vector.memset(w2[:, :], 2.0)

            # chunked fp16 loads (HWDGE) overlapped with the S/D TTs; the
            # final chunk is small so its exposed completion latency (~2us
            # sem receipt) costs little on the critical path
            bounds = [0, 1280, 2560, 3840, FREE]
            for c in range(len(bounds) - 1):
                lo, hi = bounds[c], bounds[c + 1]
                nc.sync.dma_start(out=sr_t[:, lo:hi], in_=x_sr[:, lo:hi])
                nc.sync.dma_start(out=hr_t[:, lo:hi], in_=x_hr[:, lo:hi])
            for c in range(len(bounds) - 1):
                lo, hi = bounds[c], bounds[c + 1]
                nc.vector.tensor_tensor(SD[:, lo:hi], sr_t[:, lo:hi],
                                        hr_t[:, lo:hi], Alu.add)
                nc.vector.tensor_tensor(SD[:, FREE + lo:FREE + hi],
                                        sr_t[:, lo:hi], hr_t[:, lo:hi],
                                        Alu.subtract)
            # shifted copy SDo = SD[:, 1:] on the (idle) DMA engines,
            # chunked to chase the S/D TT chunks off the critical path.
            # Chunk c of each segment reads only SD chunk c (bounds-1
            # alignment); the seam element SDo[:,4095] is junk, never read.
            for seg in (0, FREE):
                cuts = [seg] + [seg + b - 1 for b in bounds[1:]]
                for c in range(len(cuts) - 1):
                    nc.sync.dma_start(out=SDo[:, cuts[c]:cuts[c + 1]],
                                      in_=SD[:, cuts[c] + 1:cuts[c + 1] + 1])

            SDv = SD.rearrange("p (s f) -> p s f", s=2)
            SDov = SDo.rearrange("p (s f) -> p s f", s=2)

            # (offset, op window lo, abs engine, PE plan) in issue order.
            # PE plan entries: ("rows", j_lo, j_hi, row_weights) for the 15
            # weighted row matmuls, ("strip_i", j) / rows ranges for edges.
            def rows_w(nlo, nhi):
                # weight per row i in 0..14: 1 on the single-window edge
                # rows, 2 in the shared middle
                return [((1.0 if (i == 0 or i == 14) else 2.0))
                        for i in range(15)]

            PAIRS = [
                # o=256 {N,S}: windows rows 1..14 and 0..13, j 1..14 both
                (256, 0, "dve",
                 [("mid", 1, 15, rows_w(0, 15), 0, 15)]),
                # o=255 {NE,SW}: I j 1..14; I-255 rows-1, j 2..15
                (255, 0, "act",
                 [("mid", 2, 15, rows_w(0, 15), 0, 15),
                  ("strip", 1, 1, 15),     # I edge col j=1, rows 1..14
                  ("strip", 15, 0, 14)]),  # I-255 edge col j=15, rows 0..13
                # o=257 {NW,SE}: I j 1..14; I-257 rows-1, j 0..13
                (257, 0, "act",
                 [("mid", 1, 14, rows_w(0, 15), 0, 15),
                  ("strip", 14, 1, 15),    # I edge col j=14, rows 1..14
                  ("strip", 0, 0, 14)]),   # I-257 edge col j=0, rows 0..13
                # E (o=1, weight 2): rows 1..14, j 1..14 only
                (1, WC, "act",
                 [("emid", 1, 15, None, 1, 15)]),
            ]

            first_mm = [True]

            def mm(rhs, wts, stop=False):
                width = int(np.prod(rhs.shape[1:]))
                nc.tensor.matmul(acc[:, 0:width], wts[:, :], rhs,
                                 start=first_mm[0], stop=stop)
                first_mm[0] = False

            n_pairs = len(PAIRS)
            for pi, (o, oplo, abs_eng, plan) in enumerate(PAIRS):
                pq = pq_pool.tile([128, PQW], fp16, tag="pq")
                last_pair = pi == n_pairs - 1
                if last_pair:
                    # split the final pair's t into two tiles so PE can
                    # start the tail matmuls after the first min half
                    t_a = t_pool.tile([128, 2048], fp16, tag="ta")
                    t_b = t_pool.tile([128, PADW - 2048], fp16, tag="tb")
                else:
                    t = t_pool.tile([128, PADW], fp16, tag="t")
                pqv = pq.rearrange("p (s f) -> p s f", s=2)
                if o % 2 == 0:
                    src = SDv[:, :, o + oplo:o + WIN]
                else:
                    src = SDov[:, :, o - 1 + oplo:o - 1 + WIN]
                nc.vector.tensor_tensor(pqv[:, :, oplo:WIN],
                                        SDv[:, :, oplo:WIN], src,
                                        Alu.subtract)
                # |x|: sign-bit clear on DVE for the low columns, ACT Abs
                # for a slice sized to hide under the next pair's subtract
                SPLIT = WIN - 2048
                pqi = pqv[:, :, oplo:SPLIT].bitcast(mybir.dt.int16)
                nc.vector.tensor_scalar(out=pqi, in0=pqi, scalar1=0x7FFF,
                                        scalar2=None, op0=Alu.bitwise_and)
                nc.scalar.activation(pqv[:, :, SPLIT:WIN],
                                     pqv[:, :, SPLIT:WIN], Act.Abs)
                if last_pair:
                    nc.vector.tensor_tensor(
                        t_a[:, oplo:2048], pq[:, oplo:2048],
                        pq[:, PADW + oplo:PADW + 2048], Alu.min)
                    nc.vector.tensor_tensor(
                        t_b[:, 0:WIN - 2048], pq[:, 2048:WIN],
                        pq[:, PADW + 2048:PADW + WIN], Alu.min)
                    vza = t_a[:, 0:2048].rearrange("p (i q j) -> p i q j",
                                                   q=16, j=16)
                    vzb = t_b[:, 0:1792].rearrange("p (i q j) -> p i q j",
                                                   q=16, j=16)
                    vrow = lambda i: vza[:, i] if i < 8 else vzb[:, i - 8]
                else:
                    nc.vector.tensor_tensor(t[:, oplo:WIN], pq[:, oplo:WIN],
                                            pq[:, PADW + oplo:PADW + WIN],
                                            Alu.min)
                    vz = t[:, 0:WIN].rearrange("p (i q j) -> p i q j",
                                               q=16, j=16)
                    vrow = lambda i: vz[:, i]
                # PE interior reductions: weighted row matmuls into acc
                for e in plan:
                    kind, a, b = e[0], e[1], e[2]
                    if kind == "mid":
                        wts, rlo, rhi = e[3], e[4], e[5]
                        for i in range(rlo, rhi):
                            w = w1 if wts[i] == 1.0 else w2
                            mm(vrow(i)[:, :, a:b], w)
                    elif kind == "emid":
                        rlo, rhi = e[4], e[5]
                        for i in range(rlo, rhi):
                            mm(vrow(i)[:, :, a:b], w2,
                               stop=last_pair and i == rhi - 1)
                    else:  # ("strip", j_col, row_lo, row_hi)
                        mm(vz[:, b:e[3], :, a:a + 1], w1)

            # drain PSUM to a scalar
            nc.vector.tensor_reduce(colsb[:, 0:1], acc[:, 0:224],
                                    mybir.AxisListType.X, Alu.add)
            nc.sync.dma_start(out=out_sum[:, :], in_=colsb[:, :])
    _split_multiwaits(nc)
    return nc


_NC_CACHE = None
LAST_RESULTS = None  # BassKernelResults of the most recent run (for test.py)


def kernel(sr_tensor: np.ndarray, hr_tensor: np.ndarray) -> np.ndarray:
    from concourse.bass_utils import run_bass_kernel_spmd

    global _NC_CACHE, LAST_RESULTS
    if _NC_CACHE is None:
        _NC_CACHE = _build_bass()
    nc = _NC_CACHE

    # fp16 staging: the kernel computes in fp16 on-device either way; the
    # cast here just halves DMA traffic.
    sr = np.asarray(sr_tensor, dtype=np.float32).reshape(H, W)
    hr = np.asarray(hr_tensor, dtype=np.float32).reshape(H, W)

    in_maps = []
    for c in range(NCORES):
        c0 = c * WC
        # [2048, 256] -> [128 patch-rows, 16 rows, 256 cols] -> [128, 4096]
        slab_sr = np.ascontiguousarray(
            sr[:, c0:c0 + WC].reshape(128, K, WC).reshape(128, FREE)
            .astype(np.float16))
        slab_hr = np.ascontiguousarray(
            hr[:, c0:c0 + WC].reshape(128, K, WC).reshape(128, FREE)
            .astype(np.float16))
        in_maps.append({"x_sr": slab_sr, "x_hr": slab_hr})

    res = run_bass_kernel_spmd(nc, in_maps, list(range(NCORES)))
    LAST_RESULTS = res

    total = 0.0
    for r in res.results:
        total += float(np.asarray(r["out_sum"], dtype=np.float64)[0, 0])
    return np.float32(total / N_TERMS)



# revision 4
# speedup vs baseline: 1.0307x; 1.0307x over previous
"""Trainium2 Bass kernel for nn_DistanceLoss (patch neighbor-distance loss).

Reference semantics (k=16, H=W=2048, LOSS_WEIGHT=1):
  split each image into non-overlapping 16x16 patches; for interior pixels
  (local i,j in 1..14) and the 8-neighbor offset list [E,NW,NE,N,E,SW,SE,S]
  (E twice, W missing), accumulate || |sr_c-sr_n| - |hr_c-hr_n| || and take
  the global mean over L*14*14*8 terms.

Identity: for u = sr_c-sr_n, v = hr_c-hr_n,
    ||u|-|v|| = min(|u+v|, |u-v|) = min(|S_c-S_n|, |D_c-D_n|)
with S = sr+hr, D = sr-hr. Opposite offsets +o/-o share one difference
array t: sum_I t(f,-o) = sum_{I-o} t(g,+o), so the pairs {N,S}, {NW,SE},
{NE,SW} cost one elementwise pass each; E (listed twice) has weight 2.

Sharding: 256 image columns per core (16 patch-cols x 128 patch-rows).
Host reshapes each slab to [128, 4096] (partition = patch-row, free =
i*256+c) making every neighbor offset the constant free shift di*256+dj.

Measured-HW design notes (bench on the target trn2):
  - DVE TT fp16 runs 2x even with ODD element offsets (2153ns vs 2158ns
    aligned at FD 3840), so the shifted operands SD[o:...] are sliced
    directly; no SBUF->SBUF shifted-copy DMA at all (saves 2MB DMA and
    its critical-path serialization).
  - STT/TensorReduce run at 1x -> no fused accumulate paths; reductions
    stay on the otherwise-idle PE as ones/twos-weighted [128,1]^T @ t-row
    matmuls into one PSUM region (row weights {1,2,...,2,1} encode both
    shifted windows of an offset pair, strips are edge columns, E bakes
    its x2).
  - abs: ACT Abs (0.87ns/elem) takes the three 256/255/257 pairs
    (in-place on the stacked p|q tile, one-shot FD 7680); the E pair's
    abs rides DVE int16 sign-clear at 4x. This balances DVE ~31us /
    ACT ~21us.
  - input DMA: fp16, 5 chunks/tensor on parallel queues, small first
    chunk so S|D prep (chunked, stacked 2-wide TT) starts early.
"""

import numpy as np

H = W = 2048
K = 16
NCORES = 8
WC = W // NCORES          # 256 columns per core
FREE = K * WC             # 4096 free elements per partition
WIN = 15 * WC             # 3840: compute window covers i = 0..14
SEG = FREE + 64           # per-segment pad: o=257 shifted reads end at 4097
N_TERMS = (H // K) * (W // K) * (K - 2) * (K - 2) * 8


def _split_multiwaits(nc):
    """The walrus build here accepts at most one sync wait (and one update)
    per instruction: hoist extra waits onto same-engine NoOps inserted
    before the instruction, and extra updates onto NoOps after it."""
    from concourse import mybir

    k = 0
    for f in nc.m.functions:
        for bb in f.blocks:
            out, changed = [], False
            for i in bb.instructions:
                si = i.sync_info
                waits = list(si.on_wait) if si else []
                ups = list(si.on_update) if si else []
                trimmed = False
                if len(waits) > 1:
                    for w in waits[:-1]:
                        n = mybir.InstNoOp(name=f"{i.name}-sw{k}", ins=[],
                                           outs=[])
                        k += 1
                        n.engine = i.engine
                        n.sync_info = mybir.SyncInfo(on_wait=[w], on_update=[])
                        out.append(n)
                    waits, changed, trimmed = waits[-1:], True, True
                out.append(i)
                if len(ups) > 1:
                    i.sync_info = mybir.SyncInfo(on_wait=waits,
                                                 on_update=ups[:1])
                    for u in ups[1:]:
                        n = mybir.InstNoOp(name=f"{i.name}-su{k}", ins=[],
                                           outs=[])
                        k += 1
                        n.engine = i.engine
                        n.sync_info = mybir.SyncInfo(on_wait=[], on_update=[u])
                        out.append(n)
                    changed = True
                elif trimmed:
                    i.sync_info = mybir.SyncInfo(on_wait=waits, on_update=ups)
            if changed:
                bb.instructions = out
    return k


def _build_bass(debug=False):
    from concourse import bass, mybir, tile

    nc = bass.Bass()
    x_sr = nc.declare_dram_parameter("x_sr", [128, FREE], mybir.dt.float16,
                                     isOutput=False)
    x_hr = nc.declare_dram_parameter("x_hr", [128, FREE], mybir.dt.float16,
                                     isOutput=False)
    out_sum = nc.declare_dram_parameter("out_sum", [1, 8],
                                        mybir.dt.float32, isOutput=True)
    dbg_t = None
    if debug:
        dbg_t = [nc.declare_dram_parameter(f"dbg_t{k}", [128, WIN],
                                           mybir.dt.float16, isOutput=True)
                 for k in range(4)]

    fp16 = mybir.dt.float16
    f32 = mybir.dt.float32
    Alu = mybir.AluOpType
    Act = mybir.ActivationFunctionType

    with tile.TileContext(nc) as tc:
        with tc.tile_pool(name="io", bufs=1) as io_pool, \
             tc.tile_pool(name="sd", bufs=1) as sd_pool, \
             tc.tile_pool(name="pq", bufs=3) as pq_pool, \
             tc.tile_pool(name="tpool", bufs=4) as t_pool, \
             tc.tile_pool(name="psum", bufs=1, space="PSUM") as psum_pool:
            sr_t = io_pool.tile([128, FREE], fp16, tag="sr")
            hr_t = io_pool.tile([128, FREE], fp16, tag="hr")
            SD = sd_pool.tile([128, 2 * SEG], fp16, tag="SD")
            w1 = sd_pool.tile([128, 1], fp16, tag="w1")
            w2 = sd_pool.tile([128, 1], fp16, tag="w2")
            acc = psum_pool.tile([1, 256], f32, tag="acc")
            colsb = sd_pool.tile([1, 8], f32, tag="colsb")

            SDv = SD.rearrange("p (s f) -> p s f", s=2)

            nc.vector.memset(w1[:, :], 1.0)
            nc.vector.memset(w2[:, :], 2.0)
            # shifted reads run into the per-segment pad; keep it defined
            nc.vector.memset(SDv[:, :, FREE:], 0.0)

            # chunked fp16 input loads on parallel queues; small first
            # chunk so prep starts early
            bounds = [0, 640, 1536, 2432, 3264, FREE]
            for c in range(len(bounds) - 1):
                lo, hi = bounds[c], bounds[c + 1]
                nc.sync.dma_start(out=sr_t[:, lo:hi], in_=x_sr[:, lo:hi])
                nc.sync.dma_start(out=hr_t[:, lo:hi], in_=x_hr[:, lo:hi])
            # S|D prep per chunk (S=sr+hr, D=sr-hr)
            for c in range(len(bounds) - 1):
                lo, hi = bounds[c], bounds[c + 1]
                nc.vector.tensor_tensor(SDv[:, 0, lo:hi], sr_t[:, lo:hi],
                                        hr_t[:, lo:hi], Alu.add)
                nc.vector.tensor_tensor(SDv[:, 1, lo:hi], sr_t[:, lo:hi],
                                        hr_t[:, lo:hi], Alu.subtract)

            # (offset, window lo, abs engine, PE plan) in issue order.
            # PE plan entries: ("mid", j_lo, j_hi, row_weights, i_lo, i_hi)
            # for the 15 weighted row matmuls; ("strip", j, row_lo, row_hi)
            # for single-window edge columns; ("emid", ...) weight-2 rows.
            def rows_w():
                return [(1.0 if (i == 0 or i == 14) else 2.0)
                        for i in range(15)]

            PAIRS = [
                # o=256 {N,S}: windows rows 1..14 and 0..13, j 1..14 both
                (256, 0, "act", [("mid", 1, 15, rows_w(), 0, 15)]),
                # o=255 {NE,SW}: SW i 1..14 j 1..14; NE i' 0..13 j' 2..15
                (255, 0, "act",
                 [("mid", 2, 15, rows_w(), 0, 15),
                  ("strip", 1, 1, 15),     # SW-only edge col j=1, rows 1..14
                  ("strip", 15, 0, 14)]),  # NE-only edge col j=15, rows 0..13
                # o=257 {NW,SE}: SE i 1..14 j 1..14; NW i' 0..13 j' 0..13
                (257, 0, "act",
                 [("mid", 1, 14, rows_w(), 0, 15),
                  ("strip", 14, 1, 15),    # SE-only edge col j=14, rows 1..14
                  ("strip", 0, 0, 14)]),   # NW-only edge col j=0, rows 0..13
                # E (o=1, weight 2): rows 1..14, j 1..14 only
                (1, WC, "dve", [("emid", 1, 15, None, 1, 15)]),
            ]

            first_mm = [True]

            def mm(rhs, wts, stop=False):
                width = int(np.prod(rhs.shape[1:]))
                nc.tensor.matmul(acc[:, 0:width], wts[:, :], rhs,
                                 start=first_mm[0], stop=stop)
                first_mm[0] = False

            n_pairs = len(PAIRS)
            for pi, (o, oplo, abs_eng, plan) in enumerate(PAIRS):
                pq = pq_pool.tile([128, 2 * WIN], fp16, tag="pq")
                last_pair = pi == n_pairs - 1
                if last_pair:
                    # split the final pair's t into two tiles so PE can
                    # start the tail matmuls after the first min half
                    t_a = t_pool.tile([128, 2048], fp16, tag="ta")
                    t_b = t_pool.tile([128, WIN - 2048], fp16, tag="tb")
                else:
                    t = t_pool.tile([128, WIN], fp16, tag="t")
                pqv = pq.rearrange("p (s f) -> p s f", s=2)
                # p|q = SD - SD[o:]: odd offsets slice SD directly (2x TT
                # confirmed on HW for odd element offsets)
                nc.vector.tensor_tensor(pqv[:, :, oplo:WIN],
                                        SDv[:, :, oplo:WIN],
                                        SDv[:, :, o + oplo:o + WIN],
                                        Alu.subtract)
                # |pq|: ACT Abs one-shot for the three big pairs, DVE
                # int16 sign-clear (4x) for the E pair
                if abs_eng == "act":
                    nc.scalar.activation(pqv[:, :, oplo:WIN],
                                         pqv[:, :, oplo:WIN], Act.Abs)
                else:
                    pqi = pqv[:, :, oplo:WIN].bitcast(mybir.dt.int16)
                    nc.vector.tensor_scalar(out=pqi, in0=pqi, scalar1=0x7FFF,
                                            scalar2=None,
                                            op0=Alu.bitwise_and)
                if last_pair:
                    nc.vector.tensor_tensor(
                        t_a[:, oplo:2048], pq[:, oplo:2048],
                        pq[:, WIN + oplo:WIN + 2048], Alu.min)
                    nc.vector.tensor_tensor(
                        t_b[:, 0:WIN - 2048], pq[:, 2048:WIN],
                        pq[:, WIN + 2048:2 * WIN], Alu.min)
                    vza = t_a[:, 0:2048].rearrange("p (i q j) -> p i q j",
                                                   q=16, j=16)
                    vzb = t_b[:, 0:1792].rearrange("p (i q j) -> p i q j",
                                                   q=16, j=16)
                    vrow = lambda i: vza[:, i] if i < 8 else vzb[:, i - 8]
                    vz = None
                else:
                    nc.vector.tensor_tensor(t[:, oplo:WIN], pq[:, oplo:WIN],
                                            pq[:, WIN + oplo:2 * WIN],
                                            Alu.min)
                    vz = t[:, 0:WIN].rearrange("p (i q j) -> p i q j",
                                               q=16, j=16)
                    vrow = lambda i: vz[:, i]
                if debug:
                    if last_pair:
                        nc.sync.dma_start(out=dbg_t[pi][:, 0:2048],
                                          in_=t_a[:, 0:2048])
                        nc.sync.dma_start(out=dbg_t[pi][:, 2048:WIN],
                                          in_=t_b[:, 0:WIN - 2048])
                    else:
                        nc.sync.dma_start(out=dbg_t[pi][:, :], in_=t[:, :])
                # PE interior reductions: weighted row matmuls into acc
                for e in plan:
                    kind = e[0]
                    if kind == "mid":
                        _, a, b, wts, rlo, rhi = e
                        for i in range(rlo, rhi):
                            w = w1 if wts[i] == 1.0 else w2
                            mm(vrow(i)[:, :, a:b], w)
                    elif kind == "emid":
                        _, a, b, _, rlo, rhi = e
                        for i in range(rlo, rhi):
                            mm(vrow(i)[:, :, a:b], w2,
                               stop=last_pair and i == rhi - 1)
                    else:  # ("strip", j_col, row_lo, row_hi)
                        _, j, rlo, rhi = e
                        mm(vz[:, rlo:rhi, :, j:j + 1], w1)

            # drain PSUM to a scalar
            nc.vector.tensor_reduce(colsb[:, 0:1], acc[:, 0:224],
                                    mybir.AxisListType.X, Alu.add)
            nc.sync.dma_start(out=out_sum[:, :], in_=colsb[:, :])
    _split_multiwaits(nc)
    return nc


_NC_CACHE = None
LAST_RESULTS = None  # BassKernelResults of the most recent run (for test.py)


def kernel(sr_tensor: np.ndarray, hr_tensor: np.ndarray) -> np.ndarray:
    from concourse.bass_utils import run_bass_kernel_spmd

    global _NC_CACHE, LAST_RESULTS
    if _NC_CACHE is None:
        _NC_CACHE = _build_bass()
    nc = _NC_CACHE

    # fp16 staging: the kernel computes in fp16 on-device either way; the
    # cast here just halves DMA traffic.
    sr = np.asarray(sr_tensor, dtype=np.float32).reshape(H, W)
    hr = np.asarray(hr_tensor, dtype=np.float32).reshape(H, W)

    in_maps = []
    for c in range(NCORES):
        c0 = c * WC
        # [2048, 256] -> [128 patch-rows, 16 rows, 256 cols] -> [128, 4096]
        slab_sr = np.ascontiguousarray(
            sr[:, c0:c0 + WC].reshape(128, K, WC).reshape(128, FREE)
            .astype(np.float16))
        slab_hr = np.ascontiguousarray(
            hr[:, c0:c0 + WC].reshape(128, K, WC).reshape(128, FREE)
            .astype(np.float16))
        in_maps.append({"x_sr": slab_sr, "x_hr": slab_hr})

    res = run_bass_kernel_spmd(nc, in_maps, list(range(NCORES)))
    LAST_RESULTS = res

    total = 0.0
    for r in res.results:
        total += float(np.asarray(r["out_sum"], dtype=np.float64)[0, 0])
    return np.float32(total / N_TERMS)


# revision 5
# speedup vs baseline: 1.0945x; 1.0618x over previous
"""Trainium2 Bass kernel for nn_DistanceLoss (patch neighbor-distance loss).

Reference semantics (k=16, H=W=2048, LOSS_WEIGHT=1):
  split each image into non-overlapping 16x16 patches; for interior pixels
  (local i,j in 1..14) and the 8-neighbor offset list [E,NW,NE,N,E,SW,SE,S]
  (E twice, W missing), accumulate || |sr_c-sr_n| - |hr_c-hr_n| || and take
  the global mean over L*14*14*8 terms.

Identity: for u = sr_c-sr_n, v = hr_c-hr_n,
    ||u|-|v|| = min(|u+v|, |u-v|) = min(|S_c-S_n|, |D_c-D_n|)
with S = sr+hr, D = sr-hr. Opposite offsets +o/-o share one difference
array t: the pairs {N,S}, {NW,SE}, {NE,SW} cost one elementwise pass each;
E (listed twice) has weight 2.

Sharding: 256 image columns per core (16 patch-cols x 128 patch-rows).
Host reshapes each slab to [128, 4096] (partition = patch-row, free =
i*256+c) making every neighbor offset the constant free shift di*256+dj.

Measured-HW design notes (bench on the target trn2):
  - DVE TT fp16 runs 2x even with ODD element offsets, so shifted operands
    SD[o:...] are sliced directly; no SBUF->SBUF shifted-copy DMA at all.
  - STT/TensorReduce run at 1x -> no fused accumulate paths; reductions
    stay on the otherwise-idle PE as ones/twos-weighted [128,1]^T @ t-row
    matmuls into one PSUM region (row weights {1,2,...,2,1} encode both
    shifted windows of an offset pair, strips are edge columns, E bakes
    its x2). Same-weight adjacent rows batch 2-per-matmul (448 <= 512
    moving limit).
  - Everything is processed in row-halves (i rows 0..7 | 8..14): TT, abs,
    min, and the PE row-matmuls pipeline at half-tile granularity, so PE
    starts reducing a pair as soon as its first min-half lands and the
    final PE tail is only the last half of the E pair.
  - abs: ACT Abs (0.87ns/elem) takes the three 256/255/257 pairs
    (in-place halves on the stacked p|q tile); the E pair's abs rides
    DVE int16 sign-clear at 4x. Balances DVE ~27us / ACT ~20us.
  - input DMA: fp16, 6 chunks/tensor on parallel queues, small first
    chunks so S|D prep and the first pair-TT halves start early.
"""

import numpy as np

H = W = 2048
K = 16
NCORES = 8
WC = W // NCORES          # 256 columns per core
FREE = K * WC             # 4096 free elements per partition
WIN = 15 * WC             # 3840: compute window covers i = 0..14
SEG = FREE + 64           # per-segment pad: o=257 shifted reads end at 4097
HALF = 2048               # row-half split: rows 0..7 | 8..14
N_TERMS = (H // K) * (W // K) * (K - 2) * (K - 2) * 8


def _split_multiwaits(nc):
    """The walrus build here accepts at most one sync wait (and one update)
    per instruction: hoist extra waits onto same-engine NoOps inserted
    before the instruction, and extra updates onto NoOps after it."""
    from concourse import mybir

    k = 0
    for f in nc.m.functions:
        for bb in f.blocks:
            out, changed = [], False
            for i in bb.instructions:
                si = i.sync_info
                waits = list(si.on_wait) if si else []
                ups = list(si.on_update) if si else []
                trimmed = False
                if len(waits) > 1:
                    for w in waits[:-1]:
                        n = mybir.InstNoOp(name=f"{i.name}-sw{k}", ins=[],
                                           outs=[])
                        k += 1
                        n.engine = i.engine
                        n.sync_info = mybir.SyncInfo(on_wait=[w], on_update=[])
                        out.append(n)
                    waits, changed, trimmed = waits[-1:], True, True
                out.append(i)
                if len(ups) > 1:
                    i.sync_info = mybir.SyncInfo(on_wait=waits,
                                                 on_update=ups[:1])
                    for u in ups[1:]:
                        n = mybir.InstNoOp(name=f"{i.name}-su{k}", ins=[],
                                           outs=[])
                        k += 1
                        n.engine = i.engine
                        n.sync_info = mybir.SyncInfo(on_wait=[], on_update=[u])
                        out.append(n)
                    changed = True
                elif trimmed:
                    i.sync_info = mybir.SyncInfo(on_wait=waits, on_update=ups)
            if changed:
                bb.instructions = out
    return k


def _build_bass(debug=False):
    from concourse import bass, mybir, tile

    nc = bass.Bass()
    x_sr = nc.declare_dram_parameter("x_sr", [128, FREE], mybir.dt.float16,
                                     isOutput=False)
    x_hr = nc.declare_dram_parameter("x_hr", [128, FREE], mybir.dt.float16,
                                     isOutput=False)
    out_sum = nc.declare_dram_parameter("out_sum", [1, 8],
                                        mybir.dt.float32, isOutput=True)
    dbg_t = None
    if debug:
        dbg_t = [nc.declare_dram_parameter(f"dbg_t{k}", [128, WIN],
                                           mybir.dt.float16, isOutput=True)
                 for k in range(4)]

    fp16 = mybir.dt.float16
    f32 = mybir.dt.float32
    Alu = mybir.AluOpType
    Act = mybir.ActivationFunctionType

    with tile.TileContext(nc) as tc:
        with tc.tile_pool(name="io", bufs=1) as io_pool, \
             tc.tile_pool(name="sd", bufs=1) as sd_pool, \
             tc.tile_pool(name="pq", bufs=3) as pq_pool, \
             tc.tile_pool(name="tpool", bufs=4) as t_pool, \
             tc.tile_pool(name="psum", bufs=1, space="PSUM") as psum_pool:
            sr_t = io_pool.tile([128, FREE], fp16, tag="sr")
            hr_t = io_pool.tile([128, FREE], fp16, tag="hr")
            SD = sd_pool.tile([128, 2 * SEG], fp16, tag="SD")
            w1 = sd_pool.tile([128, 1], fp16, tag="w1")
            w2 = sd_pool.tile([128, 1], fp16, tag="w2")
            acc = psum_pool.tile([1, 512], f32, tag="acc")
            colsb = sd_pool.tile([1, 8], f32, tag="colsb")

            SDv = SD.rearrange("p (s f) -> p s f", s=2)

            nc.vector.memset(w1[:, :], 1.0)
            nc.vector.memset(w2[:, :], 2.0)
            # shifted reads run into the per-segment pad; keep it defined
            nc.vector.memset(SDv[:, :, FREE:], 0.0)

            # chunked fp16 input loads on parallel queues; small first
            # chunks so prep and the first pair-TT halves start early
            bounds = [0, 256, 768, 1536, 2304, 3200, FREE]
            for c in range(len(bounds) - 1):
                lo, hi = bounds[c], bounds[c + 1]
                nc.sync.dma_start(out=sr_t[:, lo:hi], in_=x_sr[:, lo:hi])
                nc.sync.dma_start(out=hr_t[:, lo:hi], in_=x_hr[:, lo:hi])
            # S|D prep per chunk (S=sr+hr, D=sr-hr)
            for c in range(len(bounds) - 1):
                lo, hi = bounds[c], bounds[c + 1]
                nc.vector.tensor_tensor(SDv[:, 0, lo:hi], sr_t[:, lo:hi],
                                        hr_t[:, lo:hi], Alu.add)
                nc.vector.tensor_tensor(SDv[:, 1, lo:hi], sr_t[:, lo:hi],
                                        hr_t[:, lo:hi], Alu.subtract)

            # Per-pair plans. Row tasks: (row, jlo, jhi, weight); strips
            # are single-window edge columns emitted as one matmul per
            # row-half. Weights {1,2,...,2,1} over rows 0..14 encode the
            # two shifted windows of each +o/-o pair; E bakes its x2.
            def midrows(jlo, jhi):
                return [(i, jlo, jhi, 1 if i in (0, 14) else 2)
                        for i in range(15)]

            PAIRS = [
                # o=256 {N,S}: rows 0..14 weighted, j 1..14
                (256, 0, "act", midrows(1, 15), []),
                # o=255 {NE,SW}: mid j 2..14 + edge cols j=1 (rows 1..14),
                # j=15 (rows 0..13)
                (255, 0, "act", midrows(2, 15), [(1, 1, 15), (15, 0, 14)]),
                # o=257 {NW,SE}: mid j 1..13 + edge cols j=14 (rows 1..14),
                # j=0 (rows 0..13)
                (257, 0, "act", midrows(1, 14), [(14, 1, 15), (0, 0, 14)]),
                # E (o=1, weight 2): rows 1..14, j 1..14
                (1, WC, "dve",
                 [(i, 1, 15, 2) for i in range(1, 15)], []),
            ]

            first_mm = [True]

            def mm(rhs, wts, stop=False):
                width = int(np.prod(rhs.shape[1:]))
                nc.tensor.matmul(acc[:, 0:width], wts[:, :], rhs,
                                 start=first_mm[0], stop=stop)
                first_mm[0] = False

            n_pairs = len(PAIRS)
            for pi, (o, oplo, abs_eng, rows, strips) in enumerate(PAIRS):
                last_pair = pi == n_pairs - 1
                pq = pq_pool.tile([128, 2 * WIN], fp16, tag="pq")
                t_a = t_pool.tile([128, HALF], fp16, tag="ta")
                t_b = t_pool.tile([128, WIN - HALF], fp16, tag="tb")
                pqv = pq.rearrange("p (s f) -> p s f", s=2)
                vza = t_a.rearrange("p (i q j) -> p i q j", q=16, j=16)
                vzb = t_b.rearrange("p (i q j) -> p i q j", q=16, j=16)

                halves = [(oplo, HALF), (HALF, WIN)]
                for hlo, hhi in halves:
                    # p|q = SD - SD[o:] (odd offsets slice SD directly;
                    # 2x TT confirmed on HW for odd element offsets)
                    nc.vector.tensor_tensor(pqv[:, :, hlo:hhi],
                                            SDv[:, :, hlo:hhi],
                                            SDv[:, :, o + hlo:o + hhi],
                                            Alu.subtract)
                for hlo, hhi in halves:
                    # |pq| in place: ACT Abs for the three big pairs,
                    # DVE int16 sign-clear (4x) for the E pair
                    if abs_eng == "act":
                        nc.scalar.activation(pqv[:, :, hlo:hhi],
                                             pqv[:, :, hlo:hhi], Act.Abs)
                    else:
                        pqi = pqv[:, :, hlo:hhi].bitcast(mybir.dt.int16)
                        nc.vector.tensor_scalar(out=pqi, in0=pqi,
                                                scalar1=0x7FFF, scalar2=None,
                                                op0=Alu.bitwise_and)
                for hi_, (hlo, hhi) in enumerate(halves):
                    # t = min(|p|, |q|) into the row-half tile
                    dst = t_a[:, hlo:hhi] if hi_ == 0 else t_b[:, 0:hhi - hlo]
                    nc.vector.tensor_tensor(dst, pq[:, hlo:hhi],
                                            pq[:, WIN + hlo:WIN + hhi],
                                            Alu.min)
                    # PE row reductions for this half, batching adjacent
                    # same-weight rows two per matmul (width <= 448)
                    hrows = [r for r in rows
                             if (r[0] < 8) == (hi_ == 0)]
                    bi = 0
                    while bi < len(hrows):
                        r0 = hrows[bi]
                        batch = [r0]
                        if (bi + 1 < len(hrows)
                                and hrows[bi + 1][0] == r0[0] + 1
                                and hrows[bi + 1][1:] == r0[1:]):
                            batch.append(hrows[bi + 1])
                        bi += len(batch)
                        i0 = r0[0] if hi_ == 0 else r0[0] - 8
                        vz = vza if hi_ == 0 else vzb
                        rhs = vz[:, i0:i0 + len(batch), :, r0[1]:r0[2]]
                        w = w1 if r0[3] == 1 else w2
                        is_last_mm = (last_pair and hi_ == 1
                                      and bi == len(hrows))
                        mm(rhs, w, stop=is_last_mm and not strips)
                    # strips for this half
                    for si, (j, rlo, rhi) in enumerate(strips):
                        lo = max(rlo, 0 if hi_ == 0 else 8)
                        hi2 = min(rhi, 8 if hi_ == 0 else 15)
                        if lo >= hi2:
                            continue
                        vz = vza if hi_ == 0 else vzb
                        base = 0 if hi_ == 0 else 8
                        mm(vz[:, lo - base:hi2 - base, :, j:j + 1], w1)
                if debug:
                    nc.sync.dma_start(out=dbg_t[pi][:, 0:HALF],
                                      in_=t_a[:, 0:HALF])
                    nc.sync.dma_start(out=dbg_t[pi][:, HALF:WIN],
                                      in_=t_b[:, 0:WIN - HALF])

            # drain PSUM to a scalar
            nc.vector.tensor_reduce(colsb[:, 0:1], acc[:, 0:448],
                                    mybir.AxisListType.X, Alu.add)
            nc.sync.dma_start(out=out_sum[:, :], in_=colsb[:, :])
    _split_multiwaits(nc)
    return nc


_NC_CACHE = None
LAST_RESULTS = None  # BassKernelResults of the most recent run (for test.py)


def kernel(sr_tensor: np.ndarray, hr_tensor: np.ndarray) -> np.ndarray:
    from concourse.bass_utils import run_bass_kernel_spmd

    global _NC_CACHE, LAST_RESULTS
    if _NC_CACHE is None:
        _NC_CACHE = _build_bass()
    nc = _NC_CACHE

    # fp16 staging: the kernel computes in fp16 on-device either way; the
    # cast here just halves DMA traffic.
    sr = np.asarray(sr_tensor, dtype=np.float32).reshape(H, W)
    hr = np.asarray(hr_tensor, dtype=np.float32).reshape(H, W)

    in_maps = []
    for c in range(NCORES):
        c0 = c * WC
        # [2048, 256] -> [128 patch-rows, 16 rows, 256 cols] -> [128, 4096]
        slab_sr = np.ascontiguousarray(
            sr[:, c0:c0 + WC].reshape(128, K, WC).reshape(128, FREE)
            .astype(np.float16))
        slab_hr = np.ascontiguousarray(
            hr[:, c0:c0 + WC].reshape(128, K, WC).reshape(128, FREE)
            .astype(np.float16))
        in_maps.append({"x_sr": slab_sr, "x_hr": slab_hr})

    res = run_bass_kernel_spmd(nc, in_maps, list(range(NCORES)))
    LAST_RESULTS = res

    total = 0.0
    for r in res.results:
        total += float(np.asarray(r["out_sum"], dtype=np.float64)[0, 0])
    return np.float32(total / N_TERMS)
